# revision 10
# baseline (speedup 1.0000x reference)
"""Trainium2 Bass kernel for nn_AttentionSeqToMasked (dense transformer attention).

Full-input contract: kernel(**inputs) takes the unsharded numpy inputs and
returns the full [B, SQ, H*D_V] float32 output.

Sharding (8 cores): data parallel over batch (B=4 -> 2 cores per batch) x
tensor parallel over heads (16 heads -> 8 per core). Each core computes
attention for one (batch, head-half) pair; host gathers the slices.

Design (vs the 403us nn baseline; 366us previous bass version):
  - Input DMAs: one 1MB DMA per (tensor, qc-chunk) from host-side
    chunk-blocked layouts, issued on one queue in strict need-order.
  - PE warmup: an accumulating dummy-matmul chain during the DMA wait keeps
    the HAM clock gate at 2.4 GHz so the first projections don't run cold.
  - Scores run row-tiled (64x128 mode): head A in SBUF partitions 0:64,
    head B in 64:128 -> the two K=64 matmuls of a pair stream concurrently.
  - AV runs col-tiled (128x64 mode): v tiles are [128, 128] (head A cols
    0:64, head B 64:128, NO ones column), the two M=64 matmuls per (pair,
    qc, kt) run concurrently into disjoint psum partition ranges of one
    [128, 512] bank. (M=65 packing is impossible: the 65th row would
    overlap col-tile B's partition range -> psum accumulate race.)
  - Softmax denominator: DVE accumulates the exp tiles (bf16 2x mode) into
    den_acc [128, 1024] per (pair, qc); two col-tiled ones[128,64]-matmuls
    then reduce over partitions into a [128, 512] psum (rows 0:64 = den_A,
    64:128 = den_B). Finalization lags 4 iters so the PE queue never waits
    on the DVE chain.
  - Projections and warm matmuls are also emitted as col-tiled M=64 halves
    (same throughput, same (128, 64) PE mode as AV/den) so each block pays
    only the 2 unavoidable mode switches around the row-tiled scores.
  - Epilogue: av psum [128, 512] (numerators, head A rows 0:64, B 64:128)
    and den rows 63:65 are copied to SBUF and DMA'd out raw; the host does
    the divide + transpose. exp: constant SHIFT=1.5 folded into the
    activation bias (cancels in the softmax ratio); mask bias per key-tile.

Scheduling: projection work for later pairs is chopped into ~0.85us psum-chunk
halves and interleaved into the attention stream as TensorE filler (3 of
every 4 blocks); qc0 AVs are deferred until xv lands, then caught up at 6
per block; the final qc block drains AVs with no lag to shorten the tail.
"""

import os
from contextlib import ExitStack

import numpy as np
import ml_dtypes

import concourse.bass as bass
import concourse.bacc as bacc
import concourse.mybir as mybir
import concourse.tile as tile
from concourse.bass_utils import run_bass_kernel_spmd

# Problem shape (hardcoded per contract)
B, SQ, SK = 4, 2048, 2048
D_PRE = 1024
H, D_QK, D_V = 16, 64, 64
N_CORES = 8
HALF = (H // 2) * D_QK  # 512 columns of the projection handled per core
N_PAIRS = 4  # head pairs per core
S_CHUNK = 512  # moving free-dim per matmul
N_DT = D_PRE // 128  # d_pre tiles of 128
N_KT = SK // 128  # key tiles of 128
N_QC = SQ // S_CHUNK  # query chunks of 512
MASK_NEG = -30000.0

# exp handling: exp(l - SHIFT) everywhere (cancels in softmax).
SHIFT = 1.5
N_ROWS_OUT = N_PAIRS * 130  # per pair: 128 numerator rows + 2 den rows
FIN_LAG = 4  # iters of lag before the den-reduce + epilogue of a (pair, qc)

F32 = mybir.dt.float32
BF16 = mybir.dt.bfloat16
BF16_NP = np.dtype(ml_dtypes.bfloat16)

_COMPILED = None


def _build_program():
    nc = bacc.Bacc("TRN2", target_bir_lowering=False, debug=False)

    # DRAM I/O (names are the in_map keys). x tensors are chunk-blocked on
    # the host ([qc][d_pre][512] dense blocks) so each chunk DMA is a fully
    # contiguous 128KB read (1KB-burst strided reads halve DMA bandwidth).
    xq = nc.dram_tensor("xq", [N_QC, D_PRE, S_CHUNK], BF16, kind="ExternalInput").ap()
    xk = nc.dram_tensor("xk", [N_QC, D_PRE, S_CHUNK], BF16, kind="ExternalInput").ap()
    xv = nc.dram_tensor("xv", [N_QC, D_PRE, S_CHUNK], BF16, kind="ExternalInput").ap()
    wq = nc.dram_tensor("wq", [D_PRE, HALF], BF16, kind="ExternalInput").ap()
    wk = nc.dram_tensor("wk", [D_PRE, HALF], BF16, kind="ExternalInput").ap()
    wv = nc.dram_tensor("wv", [D_PRE, N_PAIRS * 128], BF16, kind="ExternalInput").ap()
    bq = nc.dram_tensor("bq", [128, N_PAIRS], F32, kind="ExternalInput").ap()
    bk = nc.dram_tensor("bk", [128, N_PAIRS], F32, kind="ExternalInput").ap()
    bv = nc.dram_tensor("bv", [128, N_PAIRS * 128], F32, kind="ExternalInput").ap()
    mb = nc.dram_tensor("mb", [128, N_KT], F32, kind="ExternalInput").ap()
    out = nc.dram_tensor("out", [N_ROWS_OUT, SQ], F32, kind="ExternalOutput").ap()

    with tile.TileContext(nc) as tc:
        _emit(tc, xq, xk, xv, wq, wk, wv, bq, bk, bv, mb, out)

    nc.compile()
    return nc


def _emit(tc, xq, xk, xv, wq, wk, wv, bq, bk, bv, mb, out):
    nc = tc.nc

    with ExitStack() as ctx:
        # ---- pools ----
        xp = ctx.enter_context(tc.tile_pool(name="x", bufs=3))
        wp = ctx.enter_context(tc.tile_pool(name="w", bufs=1))
        cp = ctx.enter_context(tc.tile_pool(name="const", bufs=1))
        qkvp = ctx.enter_context(tc.tile_pool(name="qkv", bufs=1))
        expp = ctx.enter_context(tc.tile_pool(name="exp", bufs=3))
        denp = ctx.enter_context(tc.tile_pool(name="den", bufs=2))
        stgp = ctx.enter_context(tc.tile_pool(name="stg", bufs=2))

        # bufs=2: a filler chunk's first matmul must not wait for the DVE
        # bias-add of the previous chunk (the DVE queue runs ~1us deep with
        # den-accumulate work, and that wait would stall the whole PE queue)
        proj_ps = ctx.enter_context(tc.tile_pool(name="proj_ps", bufs=2, space="PSUM"))
        sc_ps = ctx.enter_context(tc.tile_pool(name="sc_ps", bufs=2, space="PSUM"))
        av_ps = ctx.enter_context(tc.tile_pool(name="av_ps", bufs=2, space="PSUM"))

        # ---- PE warmup: the HAM clock gate keeps the PE at 1.2 GHz until
        # ~3.4us of sustained activity. Burn accumulating dummy matmuls on a
        # zeroed tile during the input-DMA wait. Col-tiled M=64 halves so the
        # warm stream stays in the same (128, 64) PE mode as AV/proj.
        warm = cp.tile([128, 512], BF16, name="warm")
        nc.vector.memset(warm, 0.0)
        ones_w = cp.tile([128, 64], BF16, name="ones_w")
        nc.vector.memset(ones_w, 1.0)
        wps = proj_ps.tile([128, S_CHUNK], F32, name="warmps", tag="proj")
        warm_n = [0]

        def emit_warm(k):
            # k accumulating dummy matmul pairs (no deps, one psum tile)
            for _ in range(k):
                st = warm_n[0] == 0
                nc.tensor.matmul(
                    wps[0:64, :], lhsT=warm[:, 0:64], rhs=warm,
                    start=st, stop=False, tile_position=(0, 0),
                )
                nc.tensor.matmul(
                    wps[64:128, :], lhsT=warm[:, 64:128], rhs=warm,
                    start=st, stop=False, tile_position=(0, 64),
                )
                warm_n[0] += 1

        def end_warm():
            nc.tensor.matmul(
                wps[0:64, :], lhsT=warm[:, 0:64], rhs=warm,
                start=False, stop=True, tile_position=(0, 0),
            )
            nc.tensor.matmul(
                wps[64:128, :], lhsT=warm[:, 64:128], rhs=warm,
                start=False, stop=True, tile_position=(0, 64),
            )

        emit_warm(8)

        cp_tiles = {}

        def load_consts():
            # constants are only needed once projections produce results, so
            # their DMAs go AFTER the critical xk0/xq0 loads
            for name, ap, width in (
                ("mb_sb", mb, N_KT),
                ("bq_sb", bq, N_PAIRS),
                ("bk_sb", bk, N_PAIRS),
                ("bv_sb", bv, N_PAIRS * 128),
            ):
                t = cp.tile([128, width], F32, name=name)
                nc.sync.dma_start(t, ap)
                cp_tiles[name] = t

        # ---- streamed loads, priority-chunked: the first scores tile needs
        # only (wq, wk, xk chunk0, xq chunk0); the kt sweep then consumes the
        # rest of xk before more of xq is needed; v comes last. Each load is
        # ONE 1MB DMA on ONE queue in strict need-order (HBM bandwidth is
        # shared; parallel queues steal from the critical path). ----
        def alloc_x(pfx):
            big = xp.tile([128, N_DT * SQ], BF16, name=f"{pfx}big", tag="x")
            return big, [big[:, i * SQ : (i + 1) * SQ] for i in range(N_DT)]

        def load_x_chunk(big, xap, c, eng=None):
            dst = big.rearrange("p (dt s) -> p dt s", dt=N_DT)[
                :, :, c * S_CHUNK : (c + 1) * S_CHUNK
            ]
            src = xap[c].rearrange("(dt p) s -> p dt s", p=128)
            (eng or nc.sync).dma_start(dst, src)

        def load_w(wap, pfx, width, eng=None):
            big = wp.tile([128, N_DT * width], BF16, name=f"{pfx}big", tag=pfx)
            (eng or nc.sync).dma_start(
                big.rearrange("p (dt w) -> p dt w", dt=N_DT),
                wap.rearrange("(dt p) w -> p dt w", p=128),
            )
            return [big[:, i * width : (i + 1) * width] for i in range(N_DT)]

        xq_big, xq_sb = alloc_x("xq")
        xk_big, xk_sb = alloc_x("xk")
        xv_big, xv_sb = alloc_x("xv")
        wq_sb = load_w(wq, "wq", HALF)
        wk_sb = load_w(wk, "wk", HALF)
        load_x_chunk(xk_big, xk, 0)
        load_x_chunk(xq_big, xq, 0)
        load_consts()
        mb_sb = cp_tiles["mb_sb"]
        bq_sb = cp_tiles["bq_sb"]
        bk_sb = cp_tiles["bk_sb"]
        bv_sb = cp_tiles["bv_sb"]
        load_x_chunk(xk_big, xk, 1)
        load_x_chunk(xq_big, xq, 1)
        load_x_chunk(xk_big, xk, 2)
        wv_sb = load_w(wv, "wv", N_PAIRS * 128)
        load_x_chunk(xv_big, xv, 0)
        load_x_chunk(xk_big, xk, 3)
        load_x_chunk(xv_big, xv, 1)
        load_x_chunk(xq_big, xq, 2)
        load_x_chunk(xv_big, xv, 2)
        load_x_chunk(xv_big, xv, 3)
        load_x_chunk(xq_big, xq, 3)

        v_tiles = {}  # (pair, kt) -> [128, 128] bf16 tile
        qkT = {}  # (pfx, pair) -> [128, SQ] bf16 tile

        def qk_tile(pfx, pair):
            if (pfx, pair) not in qkT:
                qkT[(pfx, pair)] = qkvp.tile(
                    [128, SQ], BF16, name=f"{pfx}T{pair}", tag=f"{pfx}T", bufs=2
                )
            return qkT[(pfx, pair)]

        proj_ps_open = {}

        def emit_qk_chunk(pair, pfx, qc, half=None):
            # one [128, 512] projection chunk: 8 dt steps x 2 col-tiled M=64
            # matmuls + bias copy. half=0/1 emits only the first/second 4
            # contraction steps (filler granularity).
            dst = qk_tile(pfx, pair)
            w_sb = wq_sb if pfx == "q" else wk_sb
            b_sb = bq_sb if pfx == "q" else bk_sb
            x_sb = xq_sb if pfx == "q" else xk_sb
            key = (pair, pfx, qc)
            if half == 1:
                ps = proj_ps_open.pop(key)
            else:
                ps = proj_ps.tile(
                    [128, S_CHUNK], F32, name=f"{pfx}ps{pair}_{qc}", tag="proj"
                )
            dts = range(N_DT) if half is None else range(half * 4, half * 4 + 4)
            for dt_i in dts:
                rhs = x_sb[dt_i][:, qc * S_CHUNK : (qc + 1) * S_CHUNK]
                for h_i in range(2):
                    nc.tensor.matmul(
                        ps[h_i * 64 : (h_i + 1) * 64, :],
                        lhsT=w_sb[dt_i][
                            :, pair * 128 + h_i * 64 : pair * 128 + (h_i + 1) * 64
                        ],
                        rhs=rhs,
                        start=(dt_i == 0),
                        stop=(dt_i == N_DT - 1),
                        tile_position=(0, h_i * 64),
                    )
            if half == 0:
                proj_ps_open[key] = ps
            else:
                nc.vector.tensor_scalar_add(
                    dst[:, qc * S_CHUNK : (qc + 1) * S_CHUNK],
                    ps,
                    b_sb[:, pair : pair + 1],
                )

        def emit_v_chunk(g, st):
            # v projection for pairs (2g, 2g+1), one key tile: N=256, key
            # partitions col-split 64+64 so the mode stays (128, 64)
            ps = proj_ps.tile([128, S_CHUNK], F32, name=f"vps{g}_{st}", tag="proj")
            for dt_i in range(N_DT):
                rhs = wv_sb[dt_i][:, g * 256 : (g + 1) * 256]
                for h_i in range(2):
                    nc.tensor.matmul(
                        ps[h_i * 64 : (h_i + 1) * 64, 0:256],
                        lhsT=xv_sb[dt_i][
                            :, st * 128 + h_i * 64 : st * 128 + (h_i + 1) * 64
                        ],
                        rhs=rhs,
                        start=(dt_i == 0),
                        stop=(dt_i == N_DT - 1),
                        tile_position=(0, h_i * 64),
                    )
            for j in range(2):
                pair = 2 * g + j
                vt = qkvp.tile(
                    [128, 128], BF16, name=f"v{pair}_{st}", tag="v", bufs=3 * N_KT
                )
                nc.vector.tensor_add(
                    vt,
                    ps[:, j * 128 : (j + 1) * 128],
                    bv_sb[:, pair * 128 : (pair + 1) * 128],
                )
                v_tiles[(pair, st)] = vt

        # filler queue: projection chunk units (~0.85us of PE each), popped as
        # TensorE filler inside the attention stream.
        filler = []

        def _qk_half(pair, pfx, qc, half):
            return lambda: emit_qk_chunk(pair, pfx, qc, half)

        for pfx in ("q", "k"):
            for c in range(N_QC):
                filler.append(_qk_half(1, pfx, c, 0))
                filler.append(_qk_half(1, pfx, c, 1))
        filler += [(lambda st=st: emit_v_chunk(1, st)) for st in range(N_KT)]
        for pfx in ("q", "k"):
            for c in range(N_QC):
                filler.append(_qk_half(2, pfx, c, 0))
                filler.append(_qk_half(2, pfx, c, 1))
        for pfx in ("q", "k"):
            for c in range(N_QC):
                filler.append(_qk_half(3, pfx, c, 0))
                filler.append(_qk_half(3, pfx, c, 1))

        def pop_filler():
            if filler:
                filler.pop(0)()

        # prologue: pair-0 qk projections, ordered so the first scores tile
        # (needing only the qc=0 chunks of qT0/kT0) unblocks ASAP. The pair-0
        # v chunks are NOT emitted here: clumped in the prologue the
        # scheduler sinks them into one late burst (~56-62us) long after the
        # xv chunks land, and the deferred qc0 AVs then exhaust the ex ring
        # and starve ScalarE. They are instead interleaved into blocks 7-14
        # below, paced to the xv chunk DMA arrivals.
        emit_qk_chunk(0, "k", 0)
        emit_qk_chunk(0, "q", 0)
        for c in range(1, N_QC):
            emit_qk_chunk(0, "k", c)
        for c in range(1, N_QC):
            emit_qk_chunk(0, "q", c)

        # ---- software-pipelined attention stream over (pair, qc, kt) ----
        iters = [
            (pair, qc, kt)
            for pair in range(N_PAIRS)
            for qc in range(N_QC)
            for kt in range(N_KT)
        ]
        sc_map = {}
        av_map = {}
        den_map = {}
        pending_fin = []

        def emit_scores(i):
            pair, qc, kt = iters[i]
            qT = qk_tile("q", pair)
            kT = qk_tile("k", pair)
            sc = sc_ps.tile([128, 1024], F32, name=f"sc{pair}_{qc}_{kt}", tag="sc")
            # scoresT for heads A and B: row-tiled (64x128) concurrent pair
            nc.tensor.matmul(
                sc[:, 0:512],
                lhsT=kT[0:64, kt * 128 : (kt + 1) * 128],
                rhs=qT[0:64, qc * S_CHUNK : (qc + 1) * S_CHUNK],
                start=True,
                stop=True,
            )
            nc.tensor.matmul(
                sc[:, 512:1024],
                lhsT=kT[64:128, kt * 128 : (kt + 1) * 128],
                rhs=qT[64:128, qc * S_CHUNK : (qc + 1) * S_CHUNK],
                start=True,
                stop=True,
            )
            sc_map[i] = sc

        def emit_exp(i):
            # returns the bf16 [128, 1024] exp tile for iteration i
            pair, qc, kt = iters[i]
            sc = sc_map.pop(i)
            ex = expp.tile(
                [128, 1024], BF16, name=f"ex{pair}_{qc}_{kt}", tag="ex", bufs=20
            )
            nc.scalar.activation(
                ex,
                sc,
                mybir.ActivationFunctionType.Exp,
                bias=mb_sb[:, kt : kt + 1],
                scale=0.125,
            )
            return ex

        def emit_av(pair, qc, kt, ex):
            # col-tiled (128x64) concurrent AV pair: head A -> psum rows
            # 0:64, head B -> 64:128, accumulating over kt
            if kt == 0:
                av_map[(pair, qc)] = av_ps.tile(
                    [128, S_CHUNK], F32, name=f"av{pair}_{qc}", tag="av"
                )
            av = av_map[(pair, qc)]
            vt = v_tiles[(pair, kt)]
            nc.tensor.matmul(
                av[0:64, :],
                lhsT=vt[:, 0:64],
                rhs=ex[:, 0:512],
                start=(kt == 0),
                stop=(kt == N_KT - 1),
                tile_position=(0, 0),
            )
            nc.tensor.matmul(
                av[64:128, :],
                lhsT=vt[:, 64:128],
                rhs=ex[:, 512:1024],
                start=(kt == 0),
                stop=(kt == N_KT - 1),
                tile_position=(0, 64),
            )

        def emit_den(pair, qc, kt, ex):
            # DVE-side denominator partial: den_acc += ex (bf16 2x mode)
            if kt == 0:
                den_map[(pair, qc)] = denp.tile(
                    [128, 1024], BF16, name=f"den{pair}_{qc}", tag="den"
                )
                nc.vector.tensor_copy(den_map[(pair, qc)], ex)
            else:
                acc = den_map[(pair, qc)]
                nc.vector.tensor_add(acc, acc, ex)

        def finalize(pair, qc):
            # partition-reduce den_acc via two col-tiled ones-matmuls, then
            # evacuate numerators + den rows and DMA out
            av = av_map.pop((pair, qc))
            acc = den_map.pop((pair, qc))
            dps = proj_ps.tile([128, S_CHUNK], F32, name=f"dps{pair}_{qc}", tag="proj")
            nc.tensor.matmul(
                dps[0:64, :], lhsT=ones_w, rhs=acc[:, 0:512],
                start=True, stop=True, tile_position=(0, 0),
            )
            nc.tensor.matmul(
                dps[64:128, :], lhsT=ones_w, rhs=acc[:, 512:1024],
                start=True, stop=True, tile_position=(0, 64),
            )
            stg = stgp.tile([128, 512], F32, name=f"st{pair}_{qc}", tag="stg")
            nc.vector.tensor_copy(stg, av)
            # engine APs need aligned partition bases: copy the full tile
            # (DVE cost is free-size-bound anyway), DMA the 63:65 slice
            stg2 = stgp.tile([128, 512], F32, name=f"st2{pair}_{qc}", tag="stg2")
            nc.vector.tensor_copy(stg2, dps)
            nc.sync.dma_start(
                out[pair * 130 : pair * 130 + 128, qc * S_CHUNK : (qc + 1) * S_CHUNK],
                stg,
            )
            nc.sync.dma_start(
                out[
                    pair * 130 + 128 : pair * 130 + 130,
                    qc * S_CHUNK : (qc + 1) * S_CHUNK,
                ],
                stg2[63:65, :],
            )

        # Emission in 2-iteration blocks, software-pipelined:
        #   block b: exps (2b, 2b+1) | AV+den burst (2b-2, 2b-1) | scores
        #   (2b+2, 2b+3) | one filler unit. The AV inputs are always two
        #   blocks old, so the AV bursts never wait mid-stream. Iters 0..15
        #   (pair 0, qc 0) defer their AVs entirely so the exp engines start
        #   while the v projection still waits on the xv DMA.
        emit_scores(0)
        emit_scores(1)
        ex_map = {}
        n_it = len(iters)

        # pair-0 v chunks paced to the xv DMA arrivals (st 0-3 <- xv0 ~33us,
        # 4-7 <- xv1, 8-11 <- xv2, 12-15 <- xv3 ~43us; block wall ~2.1us at
        # the EXP pace starting ~22us), and the qc0 AV catch-up paced to the
        # v chunks so the ex-ring slots (bufs=20) free just ahead of the
        # EXP stream's need (EXP(i) recycles the slot of iter i-20).
        v0_sched = {7: (0, 1), 8: (2, 3), 9: (4, 5), 10: (6, 7),
                    11: (8, 9), 12: (10, 11), 13: (12, 13), 14: (14, 15)}
        catch_plan = {9: 2, 10: 2, 11: 2, 12: 2, 13: 2, 14: 4, 15: 4}

        den_q = []  # deferred den ops for this block: (pair, qc, kt, ex)

        def emit_av_i(i, defer_den=True):
            pair, qc, kt = iters[i]
            ex = ex_map.pop(i)
            emit_av(pair, qc, kt, ex)
            if defer_den:
                # den ops flush at block end so the DVE bias-adds/copies the
                # PE is waiting on aren't queued behind them
                den_q.append((pair, qc, kt, ex))
            else:
                emit_den(pair, qc, kt, ex)
            if kt == N_KT - 1:
                pending_fin.append((pair, qc))
            elif kt == FIN_LAG and pending_fin:
                finalize(*pending_fin.pop(0))

        next_av = 0
        for b in range(n_it // 2):
            i0, i1 = 2 * b, 2 * b + 1
            for i in (i0, i1):
                ex_map[i] = emit_exp(i)
            if b >= 9:
                # catch the deferred qc0 AVs up per catch_plan (paced to the
                # v chunks), then at <=6 per block. In the final qc block,
                # drain AVs with no lag to shorten the tail.
                target = 2 * b + 2 if b >= n_it // 2 - 8 else 2 * b - 2
                n_emit = min(catch_plan.get(b, 6), target - next_av)
                for _ in range(n_emit):
                    emit_av_i(next_av)
                    next_av += 1
            for st in v0_sched.get(b, ()):
                emit_v_chunk(0, st)
            if i1 + 2 < n_it:
                emit_scores(i1 + 1)
                emit_scores(i1 + 2)
            # spread the projection filler evenly (3 of every 4 blocks)
            # instead of front-loading it into pairs 0-1
            if b % 4 != 3:
                pop_filler()
            # during the input-DMA window, pad the PE queue with dummy warm
            # matmuls so DMA-wait stalls never cross the ~3.4us HAM window
            # and re-throttle the PE clock to 1.2 GHz mid-stream; from b=7
            # the interleaved v chunks keep the PE busy instead
            if b < 7:
                emit_warm(3)
            elif b == 7:
                end_warm()
            while den_q:
                emit_den(*den_q.pop(0))
        # drain the remaining AVs + epilogues
        while next_av < n_it:
            emit_av_i(next_av, defer_den=False)
            next_av += 1
        while pending_fin:
            finalize(*pending_fin.pop(0))

        assert not filler, f"{len(filler)} filler chunks left unscheduled"
        assert not ex_map and not av_map and not sc_map and not den_map


def _prep_core_inputs(pre_qs, pre_ks, pre_vs, k_mask, q_w, q_b, k_w, k_b, v_w, v_b, core):
    b = core // 2
    hh = core % 2
    cols = slice(HALF * hh, HALF * (hh + 1))

    def chunk_blocked(x):
        # [S, D_PRE] -> [N_QC, D_PRE, S_CHUNK] contiguous blocks of x^T
        xt = x.T.astype(BF16_NP)  # [D_PRE, S]
        return np.ascontiguousarray(
            xt.reshape(D_PRE, N_QC, S_CHUNK).transpose(1, 0, 2)
        )

    xq = chunk_blocked(pre_qs[b])
    xk = chunk_blocked(pre_ks[b])
    xv = chunk_blocked(pre_vs[b])
    wq = np.ascontiguousarray(q_w[:, cols]).astype(BF16_NP)
    wk = np.ascontiguousarray(k_w[:, cols]).astype(BF16_NP)
    wv = np.ascontiguousarray(v_w[:, cols]).astype(BF16_NP)

    bq = np.ascontiguousarray(q_b[cols].astype(np.float32).reshape(N_PAIRS, 128).T)
    bk = np.ascontiguousarray(k_b[cols].astype(np.float32).reshape(N_PAIRS, 128).T)
    bv_full = np.ascontiguousarray(
        np.tile(v_b[cols].astype(np.float32)[None, :], (128, 1))
    )

    # mask True -> -SHIFT, False -> MASK_NEG (exp underflows to 0)
    mrow = np.where(k_mask[b], -SHIFT, MASK_NEG).astype(np.float32)
    mb = np.ascontiguousarray(mrow.reshape(N_KT, 128).T)

    return {
        "xq": xq,
        "xk": xk,
        "xv": xv,
        "wq": wq,
        "wk": wk,
        "wv": wv,
        "bq": bq,
        "bk": bk,
        "bv": bv_full,
        "mb": mb,
    }


def kernel(pre_qs, pre_ks, pre_vs, k_mask, q_w, q_b, k_w, k_b, v_w, v_b):
    global _COMPILED
    args = (pre_qs, pre_ks, pre_vs, k_mask, q_w, q_b, k_w, k_b, v_w, v_b)
    args = tuple(np.asarray(a) for a in args)

    if _COMPILED is None:
        _COMPILED = _build_program()
    nc = _COMPILED

    in_maps = [_prep_core_inputs(*args, core=c) for c in range(N_CORES)]

    trace = bool(int(os.environ.get("BASS_KERNEL_TRACE", "0")))
    res = run_bass_kernel_spmd(
        nc,
        in_maps,
        core_ids=list(range(N_CORES)),
        trace=trace,
    )
    if trace:
        kernel.last_results = res

    out = np.empty((B, SQ, H * D_V), dtype=np.float32)
    for c in range(N_CORES):
        b = c // 2
        hh = c % 2
        r = res.results[c]["out"]  # [520, 2048] fp32
        for p in range(N_PAIRS):
            blk = r[p * 130 : (p + 1) * 130]
            for h_i in range(2):
                num = blk[h_i * 64 : (h_i + 1) * 64]  # [64, 2048]
                den = blk[128 + h_i]  # [2048]
                head = hh * 8 + 2 * p + h_i
                out[b, :, head * 64 : (head + 1) * 64] = (num / den).T
    return out
